# revision 16
# baseline (speedup 1.0000x reference)
"""BinaryFactoredLinear Trainium2 kernel.

Computes out = ((x * s2) @ sign(V)) @ sign(U).T * s1 + bias for
x [4, 4096, 4096] f32, factors [4096, 128] / [4096] — token-sharded
across 8 NeuronCores (2048 tokens each), run SPMD via
run_bass_kernel_spmd.

Default mode "b16io" (memory-bound problem -> halve HBM traffic):
host folds s2 into x (exact f32) and rounds once to bf16; the sign
matrices are +-1 so bf16 weights are exact. The kernel streams bf16
x-tiles in, accumulates both matmul stages in f32 PSUM, applies
s1/bias (exact f32 on-chip) during the PSUM->SBUF copy, and writes
bf16 outputs which the host upcasts to f32. Per-core HBM traffic is
16.8 MiB in + 16.8 MiB out (vs 33.5+33.5 for the bf16x2h mode).
Rounding error: x2->bf16, z1->bf16, out->bf16, each ~2^-9 relative;
measured end-to-end max rel err ~1e-3 against the f32 reference
(gate is 2e-2).

Host pre-tiles each core's token shard into contiguous [128, g*T]
blocks ("fat" layout) so every DMA is one fully contiguous 512 KiB+
transfer with the contraction dim on SBUF partitions. Outputs are
written the same way and reassembled on host.

Per-core pipeline (tokens tiled by T=512, all matmuls N=512 bf16):
  stage 1: z1[r=128, T] += V_sign_k.T @ x_k (32 k-chunks, one PSUM bank)
  z1 -> bf16 SBUF copy (DVE)
  stage 2: out[o*128:(o+1)*128, T] = U_sign_o @ z1
  epilogue: scale*x+bias fused into the PSUM->SBUF copy, alternating
            ScalarE activation / DVE tensor_scalar per o-group so
            neither engine is the bottleneck.
DMA queues: inputs round-robin over the SP HWDGE + gpsimd SWDGE
queues, outputs on the ACT HWDGE queue.

Tuned via For_i trip-count-diff timing on the 8 cores (see
loopbench2.py / sweep2.py): deep buffering is the lever — xbufs=12
input tiles in flight, obufs=6 output tiles, 7 PSUM banks for stage-2
plus 1 for z1 (z1 double-buffering loses to the extra stage-2 bank).
Measured 108.9 us/iter vs a 100.5 us pure-DMA-streaming probe of the
same traffic (334 GB/s/core) and 261.5 us for the bf16x2h baseline.
Rejected by measurement: deferred stage-2 ordering (152 us), ACT-only
epilogue (162), t_tile=256 (136), go=8 (122), gi=2 (113), 3-queue
spread (135), column-split epilogue (118), 5:3 ACT-weighted split
(116). gpsimd.tensor_scalar does not lower on this stack.

Mode "bf16x2h" (previous baseline, rel err ~3.5e-6): x2 split into
bf16 hi/lo pair carrying ~16 mantissa bits, f32 output.
"""

import os
from contextlib import ExitStack

import numpy as np

import concourse.bacc as bacc
import concourse.mybir as mybir
import concourse.tile as tile
from concourse.bass_utils import run_bass_kernel_spmd

F32 = mybir.dt.float32
BF16 = mybir.dt.bfloat16

B, S, D_IN, D_OUT, R = 4, 4096, 4096, 4096, 128
N_CORES = 8
TOKENS = B * S
TOK_PER_CORE = TOKENS // N_CORES

MODE = os.environ.get("BFL_MODE", "b16io")
T_TILE = int(os.environ.get("BFL_T_TILE", "512"))
G_IN = int(os.environ.get("BFL_G_IN", "4"))
G_OUT = int(os.environ.get("BFL_G_OUT", "4"))
XBUFS = int(os.environ.get("BFL_XBUFS", "12"))
OBUFS = int(os.environ.get("BFL_OBUFS", "6"))
OPBUFS = int(os.environ.get("BFL_OPBUFS", "7"))
INQ = os.environ.get("BFL_INQ", "sp,pool")
OUTQ = os.environ.get("BFL_OUTQ", "act")
EPI_SPLIT = os.environ.get("BFL_EPI_SPLIT", "alt")  # alt | act | dve
ORDER = os.environ.get("BFL_ORDER", "inline")  # inline | defer
Z1BUFS = int(os.environ.get("BFL_Z1BUFS", "1"))

Copy = mybir.ActivationFunctionType.Copy
Ident = mybir.ActivationFunctionType.Identity
SUB = mybir.AluOpType.subtract
MULT = mybir.AluOpType.mult
ADD = mybir.AluOpType.add

LOOP_HINTS = (mybir.EngineType.PE, mybir.EngineType.DVE,
              mybir.EngineType.Activation, mybir.EngineType.SP,
              mybir.EngineType.Pool)


def _q_cycler(nc, spec):
    ifaces = {"sp": nc.sync, "act": nc.scalar, "pool": nc.gpsimd}
    seq = [ifaces[s.strip()] for s in spec.split(",")]
    state = [0]

    def nxt():
        e = seq[state[0] % len(seq)]
        state[0] += 1
        return e
    return nxt


def build_b16io(d_in=D_IN, d_out=D_OUT, r=R, tok=TOK_PER_CORE,
                t_tile=T_TILE, loop=1, gi=G_IN, go=G_OUT, xbufs=XBUFS,
                obufs=OBUFS, opbufs=OPBUFS, inq=INQ, outq=OUTQ,
                epi_split=EPI_SPLIT, order=ORDER, probe=None,
                z1bufs=Z1BUFS):
    assert d_in % 128 == 0 and d_out % 128 == 0 and tok % t_tile == 0
    assert r == 128 and t_tile <= 512
    nk, no, nt = d_in // 128, d_out // 128, tok // t_tile
    assert nk % gi == 0 and no % go == 0

    nc = bacc.Bacc("TRN2", target_bir_lowering=False, debug=False)

    xt = nc.dram_tensor("xt", [nt, nk // gi, 128, gi, t_tile], BF16,
                        kind="ExternalInput")
    outt = nc.dram_tensor("outt", [nt, no // go, 128, go, t_tile], BF16,
                          kind="ExternalOutput")
    w1 = nc.dram_tensor("w1", [128, nk, r], BF16, kind="ExternalInput")
    w2 = nc.dram_tensor("w2", [r, d_out], BF16, kind="ExternalInput")
    s1c = nc.dram_tensor("s1c", [128, no], F32, kind="ExternalInput")
    biasc = nc.dram_tensor("biasc", [128, no], F32, kind="ExternalInput")

    in_dma = _q_cycler(nc, inq)
    out_dma = _q_cycler(nc, outq)

    with tile.TileContext(nc) as tc, ExitStack() as ctx:
        const = ctx.enter_context(tc.tile_pool(name="const", bufs=1))
        xpool = ctx.enter_context(tc.tile_pool(name="x", bufs=xbufs))
        z1s = ctx.enter_context(tc.tile_pool(name="z1s", bufs=2))
        osb = ctx.enter_context(tc.tile_pool(name="osb", bufs=obufs))
        z1pool = ctx.enter_context(
            tc.tile_pool(name="z1p", bufs=z1bufs, space="PSUM"))
        opsum = ctx.enter_context(
            tc.tile_pool(name="opsum", bufs=opbufs, space="PSUM"))

        w1_sb = const.tile([128, nk, r], BF16)
        nc.sync.dma_start(w1_sb[:], w1.ap())
        w2_sb = const.tile([128, d_out], BF16)
        nc.sync.dma_start(w2_sb[:], w2.ap())
        s1_sb = const.tile([128, no], F32)
        nc.sync.dma_start(s1_sb[:], s1c.ap())
        b_sb = const.tile([128, no], F32)
        nc.sync.dma_start(b_sb[:], biasc.ap())

        if probe == "dma":
            # Pure-streaming roofline probe: same in/out DMA pattern, no
            # compute. ob0 is a constant source for all output DMAs.
            ob0 = const.tile([128, go, t_tile], BF16)
            nc.vector.memset(ob0[:], 0.0)
            if loop > 1:
                ctx.enter_context(
                    tc.For_i(0, loop, 1, hint_engines=LOOP_HINTS))
            for t in range(nt):
                for kg in range(nk // gi):
                    xk = xpool.tile([128, gi, t_tile], BF16)
                    in_dma().dma_start(xk[:], xt.ap()[t, kg])
                for og in range(no // go):
                    out_dma().dma_start(outt.ap()[t, og], ob0[:])

        if loop > 1 and probe != "dma":
            ctx.enter_context(
                tc.For_i(0, loop, 1, hint_engines=LOOP_HINTS))

        def stage2_block(t, z1b):
            for og in range(no // go):
                ob = osb.tile([128, go, t_tile], BF16)
                for oi in range(go):
                    o = og * go + oi
                    op = opsum.tile([128, t_tile], F32)
                    nc.tensor.matmul(
                        op[:], w2_sb[:, o * 128:(o + 1) * 128], z1b[:],
                        start=True, stop=True)
                    if epi_split == "half":
                        # column-split each tile ACT/DVE: halves per-tile
                        # epilogue latency so PSUM banks free sooner
                        h = t_tile // 2
                        nc.scalar.activation(ob[:, oi, :h], op[:, :h],
                                             Ident, bias=b_sb[:, o:o + 1],
                                             scale=s1_sb[:, o:o + 1])
                        nc.vector.tensor_scalar(ob[:, oi, h:], op[:, h:],
                                                s1_sb[:, o:o + 1],
                                                b_sb[:, o:o + 1], MULT, ADD)
                        continue
                    if epi_split == "alt3":
                        eng = ("act", "dve", "pool")[og % 3]
                    elif epi_split == "alt":
                        eng = ("act", "dve")[og % 2]
                    elif epi_split == "w53":
                        # ACT is 1.2 GHz vs DVE 0.96, and DVE also does
                        # the z1 copy -> give ACT 5 of every 8 groups
                        eng = "act" if og % 8 in (0, 2, 4, 5, 7) else "dve"
                    else:
                        eng = epi_split
                    if eng == "act":
                        nc.scalar.activation(ob[:, oi, :], op[:], Ident,
                                             bias=b_sb[:, o:o + 1],
                                             scale=s1_sb[:, o:o + 1])
                    elif eng == "pool":
                        nc.gpsimd.tensor_scalar(ob[:, oi, :], op[:],
                                                s1_sb[:, o:o + 1],
                                                b_sb[:, o:o + 1], MULT, ADD)
                    else:
                        nc.vector.tensor_scalar(ob[:, oi, :], op[:],
                                                s1_sb[:, o:o + 1],
                                                b_sb[:, o:o + 1], MULT, ADD)
                out_dma().dma_start(outt.ap()[t, og], ob[:])

        pending = None
        for t in range(nt if probe != "dma" else 0):
            z1p = z1pool.tile([128, t_tile], F32)
            xg = {}
            for kg in range(nk // gi):
                xk = xpool.tile([128, gi, t_tile], BF16)
                in_dma().dma_start(xk[:], xt.ap()[t, kg])
                xg[kg] = xk
            for k in range(nk):
                xk = xg[k // gi][:, k % gi, :]
                nc.tensor.matmul(z1p[:], w1_sb[:, k, :], xk,
                                 start=(k == 0), stop=(k == nk - 1))

            z1b = z1s.tile([128, t_tile], BF16, tag="z1b")
            nc.vector.tensor_copy(z1b[:], z1p[:])

            if order == "defer":
                if pending is not None:
                    stage2_block(*pending)
                pending = (t, z1b)
            else:
                stage2_block(t, z1b)
        if pending is not None:
            stage2_block(*pending)

    nc.compile()
    return nc


def build_bf16x2h(d_in=D_IN, d_out=D_OUT, r=R, tok=TOK_PER_CORE,
                  t_tile=512, loop=1, g=4, xbufs=5):
    """Previous baseline: bf16 hi/lo input pair, f32 output."""
    nk, no, nt = d_in // 128, d_out // 128, tok // t_tile

    nc = bacc.Bacc("TRN2", target_bir_lowering=False, debug=False)

    xt = nc.dram_tensor("xt", [nt, nk, 128, t_tile], BF16,
                        kind="ExternalInput")
    xt2 = nc.dram_tensor("xt2", [nt, nk, 128, t_tile], BF16,
                         kind="ExternalInput")
    outt = nc.dram_tensor("outt", [nt, no, 128, t_tile], F32,
                          kind="ExternalOutput")
    w1 = nc.dram_tensor("w1", [128, nk, r], BF16, kind="ExternalInput")
    w2 = nc.dram_tensor("w2", [r, d_out], BF16, kind="ExternalInput")
    s1c = nc.dram_tensor("s1c", [128, no], F32, kind="ExternalInput")
    biasc = nc.dram_tensor("biasc", [128, no], F32, kind="ExternalInput")

    _rr = [0]

    def _dma():
        _rr[0] += 1
        return nc.sync if _rr[0] % 2 else nc.gpsimd

    with tile.TileContext(nc) as tc, ExitStack() as ctx:
        const = ctx.enter_context(tc.tile_pool(name="const", bufs=1))
        xpool = ctx.enter_context(tc.tile_pool(name="x", bufs=xbufs))
        z1s = ctx.enter_context(tc.tile_pool(name="z1s", bufs=2))
        osb = ctx.enter_context(tc.tile_pool(name="osb", bufs=3))
        z1pool = ctx.enter_context(
            tc.tile_pool(name="z1p", bufs=z1bufs, space="PSUM"))
        opsum = ctx.enter_context(
            tc.tile_pool(name="opsum", bufs=4, space="PSUM"))

        w1_sb = const.tile([128, nk, r], BF16)
        nc.sync.dma_start(w1_sb[:], w1.ap())
        w2_sb = const.tile([128, d_out], BF16)
        nc.sync.dma_start(w2_sb[:], w2.ap())
        s1_sb = const.tile([128, no], F32)
        nc.sync.dma_start(s1_sb[:], s1c.ap())
        b_sb = const.tile([128, no], F32)
        nc.sync.dma_start(b_sb[:], biasc.ap())

        if loop > 1:
            ctx.enter_context(
                tc.For_i(0, loop, 1, hint_engines=LOOP_HINTS))

        for t in range(nt):
            z1p = z1pool.tile([128, t_tile], F32)
            xg, xg2 = {}, {}
            for kg in range(nk // g):
                xk = xpool.tile([128, g, t_tile], BF16)
                _dma().dma_start(
                    xk[:], xt.ap()[t, kg * g:(kg + 1) * g].rearrange(
                        "g p s -> p g s"))
                xg[kg] = xk
                xk2 = xpool.tile([128, g, t_tile], BF16, tag="xk2",
                                 name="xk2")
                _dma().dma_start(
                    xk2[:], xt2.ap()[t, kg * g:(kg + 1) * g].rearrange(
                        "g p s -> p g s"))
                xg2[kg] = xk2
            for k in range(nk):
                xk = xg[k // g][:, k % g, :]
                xk2 = xg2[k // g][:, k % g, :]
                nc.tensor.matmul(z1p[:], w1_sb[:, k, :], xk,
                                 start=(k == 0), stop=False)
                nc.tensor.matmul(z1p[:], w1_sb[:, k, :], xk2,
                                 start=False, stop=(k == nk - 1))

            z1hi = z1s.tile([128, t_tile], BF16, tag="z1hi")
            nc.vector.tensor_copy(z1hi[:], z1p[:])
            z1lo = z1s.tile([128, t_tile], BF16, tag="z1lo")
            nc.vector.tensor_tensor(z1lo[:], z1p[:], z1hi[:], SUB)

            for og in range(no // g):
                ob = osb.tile([128, g, t_tile], F32)
                for oi in range(g):
                    o = og * g + oi
                    op = opsum.tile([128, t_tile], F32)
                    nc.tensor.matmul(
                        op[:], w2_sb[:, o * 128:(o + 1) * 128], z1hi[:],
                        start=True, stop=False)
                    nc.tensor.matmul(
                        op[:], w2_sb[:, o * 128:(o + 1) * 128], z1lo[:],
                        start=False, stop=True)
                    nc.scalar.activation(ob[:, oi, :], op[:], Ident,
                                         bias=b_sb[:, o:o + 1],
                                         scale=s1_sb[:, o:o + 1])
                _dma().dma_start(
                    outt.ap()[t, og * g:(og + 1) * g].rearrange(
                        "g p s -> p g s"), ob[:])

    nc.compile()
    return nc


def build_nc(mode=MODE, **kw):
    if mode == "b16io":
        return build_b16io(**kw)
    if mode == "bf16x2h":
        return build_bf16x2h(**kw)
    raise ValueError(mode)


def prep_inputs(x, U_latent, V_latent, s1, s2, bias, mode=MODE,
                n_cores=N_CORES, t_tile=T_TILE, gi=G_IN):
    """Host-side prep: fold s2 into x, sign + cast factors, shard tokens."""
    import ml_dtypes

    tokens = x.shape[0] * x.shape[1] if x.ndim == 3 else x.shape[0]
    d_in = x.shape[-1]
    tok_pc = tokens // n_cores
    nt, nk = tok_pc // t_tile, d_in // 128

    x2 = x.reshape(tokens, d_in) * s2[None, :]
    w1 = np.sign(V_latent).astype(np.float32)
    # pack [d_in, r] -> [128, nk, r] so the SBUF upload is contiguous
    w1 = np.ascontiguousarray(
        w1.reshape(nk, 128, -1).transpose(1, 0, 2)).astype(ml_dtypes.bfloat16)
    w2 = np.ascontiguousarray(
        np.sign(U_latent).astype(np.float32).T).astype(ml_dtypes.bfloat16)
    no = w2.shape[1] // 128
    s1c = np.ascontiguousarray(s1.reshape(no, 128).T)
    biasc = np.ascontiguousarray(bias.reshape(no, 128).T)

    if mode == "b16io":
        xb = x2.astype(ml_dtypes.bfloat16)

        def tilefmt(c):
            xs = xb[c * tok_pc:(c + 1) * tok_pc, :]
            # [nt, T, nk/gi, gi, 128] -> [nt, nk/gi, 128, gi, T]:
            # fully contiguous per DMA tile
            return np.ascontiguousarray(
                xs.reshape(nt, t_tile, nk // gi, gi, 128).transpose(
                    0, 2, 4, 3, 1))

        return [{"w1": w1, "w2": w2, "s1c": s1c, "biasc": biasc,
                 "xt": tilefmt(c)} for c in range(n_cores)]

    xhi = x2.astype(ml_dtypes.bfloat16)
    xlo = (x2 - xhi.astype(np.float32)).astype(ml_dtypes.bfloat16)

    def tilefmt2(arr2d, c):
        xs = arr2d[c * tok_pc:(c + 1) * tok_pc, :]
        # [nt, T, nk, 128] -> [nt, nk, 128, T]
        return np.ascontiguousarray(
            xs.reshape(nt, t_tile, nk, 128).transpose(0, 2, 3, 1))

    return [{"w1": w1, "w2": w2, "s1c": s1c, "biasc": biasc,
             "xt": tilefmt2(xhi, c), "xt2": tilefmt2(xlo, c)}
            for c in range(n_cores)]


def gather_out(results, mode=MODE, n_cores=N_CORES, t_tile=T_TILE, go=G_OUT):
    out = np.empty((TOKENS, D_OUT), np.float32)
    for c in range(n_cores):
        ot = results[c]["outt"]
        if mode == "b16io":
            # [nt, no/go, 128, go, T] -> [tok_pc, d_out], bf16 -> f32
            shard = ot.transpose(0, 4, 1, 3, 2).reshape(
                TOK_PER_CORE, D_OUT).astype(np.float32)
        else:
            # [nt, no, 128, T] -> [tok_pc, d_out]
            shard = ot.transpose(0, 3, 1, 2).reshape(TOK_PER_CORE, D_OUT)
        out[c * TOK_PER_CORE:(c + 1) * TOK_PER_CORE, :] = shard
    return out.reshape(B, S, D_OUT)


_NC_CACHE = {}


def run(inputs, mode=MODE, trace=False):
    if mode not in _NC_CACHE:
        _NC_CACHE[mode] = build_nc(mode=mode)
    nc = _NC_CACHE[mode]
    in_maps = prep_inputs(**inputs, mode=mode)
    res = run_bass_kernel_spmd(nc, in_maps, list(range(N_CORES)),
                               trace=trace)
    return gather_out(res.results, mode=mode), res


def kernel(**inputs):
    inputs = {k: np.asarray(v) for k, v in inputs.items()}
    out, _ = run(inputs)
    return out


# revision 20
# speedup vs baseline: 1.0284x; 1.0284x over previous
"""BinaryFactoredLinear Trainium2 kernel.

Computes out = ((x * s2) @ sign(V)) @ sign(U).T * s1 + bias for
x [4, 4096, 4096] f32, factors [4096, 128] / [4096] — token-sharded
across 8 NeuronCores (2048 tokens each), run SPMD via
run_bass_kernel_spmd.

Default mode "b16io" (memory-bound problem -> halve HBM traffic):
host folds s2 into x (exact f32) and rounds once to bf16; the sign
matrices are +-1 so bf16 weights are exact. The kernel streams bf16
x-tiles in, accumulates both matmul stages in f32 PSUM, applies
s1/bias (exact f32 on-chip) during the PSUM->SBUF copy, and writes
bf16 outputs which the host upcasts to f32. Per-core HBM traffic is
16.8 MiB in + 16.8 MiB out (vs 33.5+33.5 for the bf16x2h mode).
Rounding error: x2->bf16, z1->bf16, out->bf16, each ~2^-9 relative;
measured end-to-end max rel err ~1e-3 against the f32 reference
(gate is 2e-2).

Host pre-tiles each core's token shard into contiguous [128, g*T]
blocks ("fat" layout) so every DMA is one fully contiguous 512 KiB+
transfer with the contraction dim on SBUF partitions. Outputs are
written the same way and reassembled on host.

Per-core pipeline (tokens tiled by T=512, all matmuls N=512 bf16):
  stage 1: z1[r=128, T] += V_sign_k.T @ x_k (32 k-chunks, one PSUM bank)
  z1 -> bf16 SBUF copy (DVE)
  stage 2: out[o*128:(o+1)*128, T] = U_sign_o @ z1
  epilogue: scale*x+bias fused into the PSUM->SBUF copy, alternating
            ScalarE activation / DVE tensor_scalar per o-group so
            neither engine is the bottleneck.
DMA queues: inputs round-robin over the SP HWDGE + gpsimd SWDGE
queues, outputs on the ACT HWDGE queue.

Tuned via For_i trip-count-diff timing on the 8 cores (see
loopbench2.py / sweep2.py): deep buffering is the lever — xbufs=16
input tiles in flight (2 full t-tiles, rotation-aligned), obufs=8
output tiles (1 t-tile of groups), 7 PSUM banks for stage-2 plus 1
for z1 (z1 double-buffering loses to the extra stage-2 bank).
Measured 108.1 us/iter vs a 100.5 us pure-DMA-streaming probe of the
same traffic (334 GB/s/core) and 261.5 us for the bf16x2h baseline.
Rejected by measurement: deferred stage-2 ordering (152 us), ACT-only
epilogue (162), t_tile=256 (136), go=8 (122), gi=2 (113), 3-queue
spread (135), column-split epilogue (118), 5:3 ACT-weighted split
(116). gpsimd.tensor_scalar does not lower on this stack.

Mode "bf16x2h" (previous baseline, rel err ~3.5e-6): x2 split into
bf16 hi/lo pair carrying ~16 mantissa bits, f32 output.
"""

import os
from contextlib import ExitStack

import numpy as np

import concourse.bacc as bacc
import concourse.mybir as mybir
import concourse.tile as tile
from concourse.bass_utils import run_bass_kernel_spmd

F32 = mybir.dt.float32
BF16 = mybir.dt.bfloat16

B, S, D_IN, D_OUT, R = 4, 4096, 4096, 4096, 128
N_CORES = 8
TOKENS = B * S
TOK_PER_CORE = TOKENS // N_CORES

MODE = os.environ.get("BFL_MODE", "b16io")
T_TILE = int(os.environ.get("BFL_T_TILE", "512"))
G_IN = int(os.environ.get("BFL_G_IN", "4"))
G_OUT = int(os.environ.get("BFL_G_OUT", "4"))
XBUFS = int(os.environ.get("BFL_XBUFS", "16"))
OBUFS = int(os.environ.get("BFL_OBUFS", "8"))
OPBUFS = int(os.environ.get("BFL_OPBUFS", "7"))
INQ = os.environ.get("BFL_INQ", "sp,pool")
OUTQ = os.environ.get("BFL_OUTQ", "act")
EPI_SPLIT = os.environ.get("BFL_EPI_SPLIT", "alt")  # alt | act | dve
ORDER = os.environ.get("BFL_ORDER", "inline")  # inline | defer
Z1BUFS = int(os.environ.get("BFL_Z1BUFS", "1"))

Copy = mybir.ActivationFunctionType.Copy
Ident = mybir.ActivationFunctionType.Identity
SUB = mybir.AluOpType.subtract
MULT = mybir.AluOpType.mult
ADD = mybir.AluOpType.add

LOOP_HINTS = (mybir.EngineType.PE, mybir.EngineType.DVE,
              mybir.EngineType.Activation, mybir.EngineType.SP,
              mybir.EngineType.Pool)


def _q_cycler(nc, spec):
    ifaces = {"sp": nc.sync, "act": nc.scalar, "pool": nc.gpsimd}
    seq = [ifaces[s.strip()] for s in spec.split(",")]
    state = [0]

    def nxt():
        e = seq[state[0] % len(seq)]
        state[0] += 1
        return e
    return nxt


def build_b16io(d_in=D_IN, d_out=D_OUT, r=R, tok=TOK_PER_CORE,
                t_tile=T_TILE, loop=1, gi=G_IN, go=G_OUT, xbufs=XBUFS,
                obufs=OBUFS, opbufs=OPBUFS, inq=INQ, outq=OUTQ,
                epi_split=EPI_SPLIT, order=ORDER, probe=None,
                z1bufs=Z1BUFS, pewarm=0):
    assert d_in % 128 == 0 and d_out % 128 == 0 and tok % t_tile == 0
    assert r == 128 and t_tile <= 512
    nk, no, nt = d_in // 128, d_out // 128, tok // t_tile
    assert nk % gi == 0 and no % go == 0

    nc = bacc.Bacc("TRN2", target_bir_lowering=False, debug=False)

    xt = nc.dram_tensor("xt", [nt, nk // gi, 128, gi, t_tile], BF16,
                        kind="ExternalInput")
    outt = nc.dram_tensor("outt", [nt, no // go, 128, go, t_tile], BF16,
                          kind="ExternalOutput")
    w1 = nc.dram_tensor("w1", [128, nk, r], BF16, kind="ExternalInput")
    w2 = nc.dram_tensor("w2", [r, d_out], BF16, kind="ExternalInput")
    s1c = nc.dram_tensor("s1c", [128, no], F32, kind="ExternalInput")
    biasc = nc.dram_tensor("biasc", [128, no], F32, kind="ExternalInput")

    in_dma = _q_cycler(nc, "sp,pool" if inq == "blk" else inq)
    out_dma = _q_cycler(nc, outq)

    with tile.TileContext(nc) as tc, ExitStack() as ctx:
        const = ctx.enter_context(tc.tile_pool(name="const", bufs=1))
        xpool = ctx.enter_context(tc.tile_pool(name="x", bufs=xbufs))
        z1s = ctx.enter_context(tc.tile_pool(name="z1s", bufs=2))
        osb = ctx.enter_context(tc.tile_pool(name="osb", bufs=obufs))
        z1pool = ctx.enter_context(
            tc.tile_pool(name="z1p", bufs=z1bufs, space="PSUM"))
        opsum = ctx.enter_context(
            tc.tile_pool(name="opsum", bufs=opbufs, space="PSUM"))

        w1_sb = const.tile([128, nk, r], BF16)
        nc.sync.dma_start(w1_sb[:], w1.ap())
        w2_sb = const.tile([128, d_out], BF16)
        nc.sync.dma_start(w2_sb[:], w2.ap())
        s1_sb = const.tile([128, no], F32)
        nc.sync.dma_start(s1_sb[:], s1c.ap())
        b_sb = const.tile([128, no], F32)
        nc.sync.dma_start(b_sb[:], biasc.ap())

        if probe == "dma":
            # Pure-streaming roofline probe: same in/out DMA pattern, no
            # compute. ob0 is a constant source for all output DMAs.
            ob0 = const.tile([128, go, t_tile], BF16)
            nc.vector.memset(ob0[:], 0.0)
            if loop > 1:
                ctx.enter_context(
                    tc.For_i(0, loop, 1, hint_engines=LOOP_HINTS))
            for t in range(nt):
                for kg in range(nk // gi):
                    xk = xpool.tile([128, gi, t_tile], BF16)
                    in_dma().dma_start(xk[:], xt.ap()[t, kg])
                for og in range(no // go):
                    out_dma().dma_start(outt.ap()[t, og], ob0[:])

        if loop > 1 and probe != "dma":
            ctx.enter_context(
                tc.For_i(0, loop, 1, hint_engines=LOOP_HINTS))

        def stage2_block(t, z1b):
            for og in range(no // go):
                ob = osb.tile([128, go, t_tile], BF16)
                for oi in range(go):
                    o = og * go + oi
                    op = opsum.tile([128, t_tile], F32)
                    if og == 0 and oi == 0:
                        # p-state bridge: dep-free dummy matmuls keep PE
                        # busy through the z1-copy gap so it holds its
                        # ramped clock; the real matmul below resets the
                        # accumulator with start=True.
                        for _ in range(pewarm):
                            nc.tensor.matmul(op[:], w2_sb[:, 0:128],
                                             w2_sb[:, :t_tile],
                                             start=True, stop=False)
                    nc.tensor.matmul(
                        op[:], w2_sb[:, o * 128:(o + 1) * 128], z1b[:],
                        start=True, stop=True)
                    if epi_split == "half":
                        # column-split each tile ACT/DVE: halves per-tile
                        # epilogue latency so PSUM banks free sooner
                        h = t_tile // 2
                        nc.scalar.activation(ob[:, oi, :h], op[:, :h],
                                             Ident, bias=b_sb[:, o:o + 1],
                                             scale=s1_sb[:, o:o + 1])
                        nc.vector.tensor_scalar(ob[:, oi, h:], op[:, h:],
                                                s1_sb[:, o:o + 1],
                                                b_sb[:, o:o + 1], MULT, ADD)
                        continue
                    if epi_split == "alt3":
                        eng = ("act", "dve", "pool")[og % 3]
                    elif epi_split == "alt":
                        eng = ("act", "dve")[og % 2]
                    elif epi_split == "w53":
                        # ACT is 1.2 GHz vs DVE 0.96, and DVE also does
                        # the z1 copy -> give ACT 5 of every 8 groups
                        eng = "act" if og % 8 in (0, 2, 4, 5, 7) else "dve"
                    else:
                        eng = epi_split
                    if eng == "act":
                        nc.scalar.activation(ob[:, oi, :], op[:], Ident,
                                             bias=b_sb[:, o:o + 1],
                                             scale=s1_sb[:, o:o + 1])
                    elif eng == "pool":
                        nc.gpsimd.tensor_scalar(ob[:, oi, :], op[:],
                                                s1_sb[:, o:o + 1],
                                                b_sb[:, o:o + 1], MULT, ADD)
                    else:
                        nc.vector.tensor_scalar(ob[:, oi, :], op[:],
                                                s1_sb[:, o:o + 1],
                                                b_sb[:, o:o + 1], MULT, ADD)
                out_dma().dma_start(outt.ap()[t, og], ob[:])

        pending = None
        for t in range(nt if probe != "dma" else 0):
            z1p = z1pool.tile([128, t_tile], F32)
            xg = {}
            ngr = nk // gi
            for kg in range(ngr):
                xk = xpool.tile([128, gi, t_tile], BF16)
                if inq == "blk":
                    # PE consumes chunks in order: serve the first half of
                    # each tile from the fast SP HWDGE queue, prefetch the
                    # second half on the Pool SWDGE queue whose gen latency
                    # is then hidden behind the earlier chunks.
                    eng = nc.sync if kg < ngr // 2 else nc.gpsimd
                else:
                    eng = in_dma()
                eng.dma_start(xk[:], xt.ap()[t, kg])
                xg[kg] = xk
            for k in range(nk):
                xk = xg[k // gi][:, k % gi, :]
                nc.tensor.matmul(z1p[:], w1_sb[:, k, :], xk,
                                 start=(k == 0), stop=(k == nk - 1))

            z1b = z1s.tile([128, t_tile], BF16, tag="z1b")
            nc.vector.tensor_copy(z1b[:], z1p[:])

            if order == "defer":
                if pending is not None:
                    stage2_block(*pending)
                pending = (t, z1b)
            else:
                stage2_block(t, z1b)
        if pending is not None:
            stage2_block(*pending)

    nc.compile()
    return nc


def build_bf16x2h(d_in=D_IN, d_out=D_OUT, r=R, tok=TOK_PER_CORE,
                  t_tile=512, loop=1, g=4, xbufs=5):
    """Previous baseline: bf16 hi/lo input pair, f32 output."""
    nk, no, nt = d_in // 128, d_out // 128, tok // t_tile

    nc = bacc.Bacc("TRN2", target_bir_lowering=False, debug=False)

    xt = nc.dram_tensor("xt", [nt, nk, 128, t_tile], BF16,
                        kind="ExternalInput")
    xt2 = nc.dram_tensor("xt2", [nt, nk, 128, t_tile], BF16,
                         kind="ExternalInput")
    outt = nc.dram_tensor("outt", [nt, no, 128, t_tile], F32,
                          kind="ExternalOutput")
    w1 = nc.dram_tensor("w1", [128, nk, r], BF16, kind="ExternalInput")
    w2 = nc.dram_tensor("w2", [r, d_out], BF16, kind="ExternalInput")
    s1c = nc.dram_tensor("s1c", [128, no], F32, kind="ExternalInput")
    biasc = nc.dram_tensor("biasc", [128, no], F32, kind="ExternalInput")

    _rr = [0]

    def _dma():
        _rr[0] += 1
        return nc.sync if _rr[0] % 2 else nc.gpsimd

    with tile.TileContext(nc) as tc, ExitStack() as ctx:
        const = ctx.enter_context(tc.tile_pool(name="const", bufs=1))
        xpool = ctx.enter_context(tc.tile_pool(name="x", bufs=xbufs))
        z1s = ctx.enter_context(tc.tile_pool(name="z1s", bufs=2))
        osb = ctx.enter_context(tc.tile_pool(name="osb", bufs=3))
        z1pool = ctx.enter_context(
            tc.tile_pool(name="z1p", bufs=z1bufs, space="PSUM"))
        opsum = ctx.enter_context(
            tc.tile_pool(name="opsum", bufs=4, space="PSUM"))

        w1_sb = const.tile([128, nk, r], BF16)
        nc.sync.dma_start(w1_sb[:], w1.ap())
        w2_sb = const.tile([128, d_out], BF16)
        nc.sync.dma_start(w2_sb[:], w2.ap())
        s1_sb = const.tile([128, no], F32)
        nc.sync.dma_start(s1_sb[:], s1c.ap())
        b_sb = const.tile([128, no], F32)
        nc.sync.dma_start(b_sb[:], biasc.ap())

        if loop > 1:
            ctx.enter_context(
                tc.For_i(0, loop, 1, hint_engines=LOOP_HINTS))

        for t in range(nt):
            z1p = z1pool.tile([128, t_tile], F32)
            xg, xg2 = {}, {}
            for kg in range(nk // g):
                xk = xpool.tile([128, g, t_tile], BF16)
                _dma().dma_start(
                    xk[:], xt.ap()[t, kg * g:(kg + 1) * g].rearrange(
                        "g p s -> p g s"))
                xg[kg] = xk
                xk2 = xpool.tile([128, g, t_tile], BF16, tag="xk2",
                                 name="xk2")
                _dma().dma_start(
                    xk2[:], xt2.ap()[t, kg * g:(kg + 1) * g].rearrange(
                        "g p s -> p g s"))
                xg2[kg] = xk2
            for k in range(nk):
                xk = xg[k // g][:, k % g, :]
                xk2 = xg2[k // g][:, k % g, :]
                nc.tensor.matmul(z1p[:], w1_sb[:, k, :], xk,
                                 start=(k == 0), stop=False)
                nc.tensor.matmul(z1p[:], w1_sb[:, k, :], xk2,
                                 start=False, stop=(k == nk - 1))

            z1hi = z1s.tile([128, t_tile], BF16, tag="z1hi")
            nc.vector.tensor_copy(z1hi[:], z1p[:])
            z1lo = z1s.tile([128, t_tile], BF16, tag="z1lo")
            nc.vector.tensor_tensor(z1lo[:], z1p[:], z1hi[:], SUB)

            for og in range(no // g):
                ob = osb.tile([128, g, t_tile], F32)
                for oi in range(g):
                    o = og * g + oi
                    op = opsum.tile([128, t_tile], F32)
                    nc.tensor.matmul(
                        op[:], w2_sb[:, o * 128:(o + 1) * 128], z1hi[:],
                        start=True, stop=False)
                    nc.tensor.matmul(
                        op[:], w2_sb[:, o * 128:(o + 1) * 128], z1lo[:],
                        start=False, stop=True)
                    nc.scalar.activation(ob[:, oi, :], op[:], Ident,
                                         bias=b_sb[:, o:o + 1],
                                         scale=s1_sb[:, o:o + 1])
                _dma().dma_start(
                    outt.ap()[t, og * g:(og + 1) * g].rearrange(
                        "g p s -> p g s"), ob[:])

    nc.compile()
    return nc


def build_nc(mode=MODE, **kw):
    if mode == "b16io":
        return build_b16io(**kw)
    if mode == "bf16x2h":
        return build_bf16x2h(**kw)
    raise ValueError(mode)


def prep_inputs(x, U_latent, V_latent, s1, s2, bias, mode=MODE,
                n_cores=N_CORES, t_tile=T_TILE, gi=G_IN):
    """Host-side prep: fold s2 into x, sign + cast factors, shard tokens."""
    import ml_dtypes

    tokens = x.shape[0] * x.shape[1] if x.ndim == 3 else x.shape[0]
    d_in = x.shape[-1]
    tok_pc = tokens // n_cores
    nt, nk = tok_pc // t_tile, d_in // 128

    x2 = x.reshape(tokens, d_in) * s2[None, :]
    w1 = np.sign(V_latent).astype(np.float32)
    # pack [d_in, r] -> [128, nk, r] so the SBUF upload is contiguous
    w1 = np.ascontiguousarray(
        w1.reshape(nk, 128, -1).transpose(1, 0, 2)).astype(ml_dtypes.bfloat16)
    w2 = np.ascontiguousarray(
        np.sign(U_latent).astype(np.float32).T).astype(ml_dtypes.bfloat16)
    no = w2.shape[1] // 128
    s1c = np.ascontiguousarray(s1.reshape(no, 128).T)
    biasc = np.ascontiguousarray(bias.reshape(no, 128).T)

    if mode == "b16io":
        xb = x2.astype(ml_dtypes.bfloat16)

        def tilefmt(c):
            xs = xb[c * tok_pc:(c + 1) * tok_pc, :]
            # [nt, T, nk/gi, gi, 128] -> [nt, nk/gi, 128, gi, T]:
            # fully contiguous per DMA tile
            return np.ascontiguousarray(
                xs.reshape(nt, t_tile, nk // gi, gi, 128).transpose(
                    0, 2, 4, 3, 1))

        return [{"w1": w1, "w2": w2, "s1c": s1c, "biasc": biasc,
                 "xt": tilefmt(c)} for c in range(n_cores)]

    xhi = x2.astype(ml_dtypes.bfloat16)
    xlo = (x2 - xhi.astype(np.float32)).astype(ml_dtypes.bfloat16)

    def tilefmt2(arr2d, c):
        xs = arr2d[c * tok_pc:(c + 1) * tok_pc, :]
        # [nt, T, nk, 128] -> [nt, nk, 128, T]
        return np.ascontiguousarray(
            xs.reshape(nt, t_tile, nk, 128).transpose(0, 2, 3, 1))

    return [{"w1": w1, "w2": w2, "s1c": s1c, "biasc": biasc,
             "xt": tilefmt2(xhi, c), "xt2": tilefmt2(xlo, c)}
            for c in range(n_cores)]


def gather_out(results, mode=MODE, n_cores=N_CORES, t_tile=T_TILE, go=G_OUT):
    out = np.empty((TOKENS, D_OUT), np.float32)
    for c in range(n_cores):
        ot = results[c]["outt"]
        if mode == "b16io":
            # [nt, no/go, 128, go, T] -> [tok_pc, d_out], bf16 -> f32
            shard = ot.transpose(0, 4, 1, 3, 2).reshape(
                TOK_PER_CORE, D_OUT).astype(np.float32)
        else:
            # [nt, no, 128, T] -> [tok_pc, d_out]
            shard = ot.transpose(0, 3, 1, 2).reshape(TOK_PER_CORE, D_OUT)
        out[c * TOK_PER_CORE:(c + 1) * TOK_PER_CORE, :] = shard
    return out.reshape(B, S, D_OUT)


_NC_CACHE = {}


def run(inputs, mode=MODE, trace=False):
    if mode not in _NC_CACHE:
        _NC_CACHE[mode] = build_nc(mode=mode)
    nc = _NC_CACHE[mode]
    in_maps = prep_inputs(**inputs, mode=mode)
    res = run_bass_kernel_spmd(nc, in_maps, list(range(N_CORES)),
                               trace=trace)
    return gather_out(res.results, mode=mode), res


def kernel(**inputs):
    inputs = {k: np.asarray(v) for k, v in inputs.items()}
    out, _ = run(inputs)
    return out


# revision 23
# speedup vs baseline: 1.0443x; 1.0155x over previous
"""BinaryFactoredLinear Trainium2 kernel.

Computes out = ((x * s2) @ sign(V)) @ sign(U).T * s1 + bias for
x [4, 4096, 4096] f32, factors [4096, 128] / [4096] — token-sharded
across 8 NeuronCores (2048 tokens each), run SPMD via
run_bass_kernel_spmd.

Default mode "b16io" (memory-bound problem -> halve HBM traffic):
host folds s2 into x (exact f32) and rounds once to bf16; the sign
matrices are +-1 so bf16 weights are exact. The kernel streams bf16
x-tiles in, accumulates both matmul stages in f32 PSUM, applies
s1/bias (exact f32 on-chip) during the PSUM->SBUF copy, and writes
bf16 outputs which the host upcasts to f32. Per-core HBM traffic is
16.8 MiB in + 16.8 MiB out (vs 33.5+33.5 for the bf16x2h mode).
Rounding error: x2->bf16, z1->bf16, out->bf16, each ~2^-9 relative;
measured end-to-end max rel err ~1e-3 against the f32 reference
(gate is 2e-2).

Host pre-tiles each core's token shard into contiguous [128, g*T]
blocks ("fat" layout) so every DMA is one fully contiguous 512 KiB+
transfer with the contraction dim on SBUF partitions. Outputs are
written the same way and reassembled on host.

Per-core pipeline (tokens tiled by T=512, all matmuls N=512 bf16):
  stage 1: z1[r=128, T] += V_sign_k.T @ x_k (32 k-chunks, one PSUM bank)
  z1 -> bf16 SBUF copy (DVE)
  stage 2: out[o*128:(o+1)*128, T] = U_sign_o @ z1
  epilogue: scale*x+bias fused into the PSUM->SBUF copy, alternating
            ScalarE activation / DVE tensor_scalar per TILE (altoi)
            so both engines finish each o-group in parallel and its
            out-DMA issues ~0.9 us sooner (106.3 vs 107.6 us for
            per-group alternation, same-process control).
DMA queues: inputs round-robin over the SP HWDGE + gpsimd SWDGE
queues, outputs on the ACT HWDGE queue.

Tuned via For_i trip-count-diff timing on the 8 cores (see
loopbench2.py / sweep2.py): deep buffering is the lever — xbufs=16
input tiles in flight (2 full t-tiles, rotation-aligned), obufs=8
output tiles (1 t-tile of groups), 7 PSUM banks for stage-2 plus 1
for z1 (z1 double-buffering loses to the extra stage-2 bank).
Measured 108.1 us/iter vs a 100.5 us pure-DMA-streaming probe of the
same traffic (334 GB/s/core) and 261.5 us for the bf16x2h baseline.
Rejected by measurement: deferred stage-2 ordering (152 us), ACT-only
epilogue (162), t_tile=256 (136), go=8 (122), gi=2 (113), 3-queue
spread (135), column-split epilogue (118), 5:3 ACT-weighted split
(116). gpsimd.tensor_scalar does not lower on this stack.

Mode "bf16x2h" (previous baseline, rel err ~3.5e-6): x2 split into
bf16 hi/lo pair carrying ~16 mantissa bits, f32 output.
"""

import os
from contextlib import ExitStack

import numpy as np

import concourse.bacc as bacc
import concourse.mybir as mybir
import concourse.tile as tile
from concourse.bass_utils import run_bass_kernel_spmd

F32 = mybir.dt.float32
BF16 = mybir.dt.bfloat16

B, S, D_IN, D_OUT, R = 4, 4096, 4096, 4096, 128
N_CORES = 8
TOKENS = B * S
TOK_PER_CORE = TOKENS // N_CORES

MODE = os.environ.get("BFL_MODE", "b16io")
T_TILE = int(os.environ.get("BFL_T_TILE", "512"))
G_IN = int(os.environ.get("BFL_G_IN", "4"))
G_OUT = int(os.environ.get("BFL_G_OUT", "4"))
XBUFS = int(os.environ.get("BFL_XBUFS", "16"))
OBUFS = int(os.environ.get("BFL_OBUFS", "8"))
OPBUFS = int(os.environ.get("BFL_OPBUFS", "7"))
INQ = os.environ.get("BFL_INQ", "sp,pool")
OUTQ = os.environ.get("BFL_OUTQ", "act")
EPI_SPLIT = os.environ.get("BFL_EPI_SPLIT", "altoi")  # altoi | alt | act | dve
ORDER = os.environ.get("BFL_ORDER", "inline")  # inline | defer
Z1BUFS = int(os.environ.get("BFL_Z1BUFS", "1"))

Copy = mybir.ActivationFunctionType.Copy
Ident = mybir.ActivationFunctionType.Identity
SUB = mybir.AluOpType.subtract
MULT = mybir.AluOpType.mult
ADD = mybir.AluOpType.add

LOOP_HINTS = (mybir.EngineType.PE, mybir.EngineType.DVE,
              mybir.EngineType.Activation, mybir.EngineType.SP,
              mybir.EngineType.Pool)


def _q_cycler(nc, spec):
    ifaces = {"sp": nc.sync, "act": nc.scalar, "pool": nc.gpsimd}
    seq = [ifaces[s.strip()] for s in spec.split(",")]
    state = [0]

    def nxt():
        e = seq[state[0] % len(seq)]
        state[0] += 1
        return e
    return nxt


def build_b16io(d_in=D_IN, d_out=D_OUT, r=R, tok=TOK_PER_CORE,
                t_tile=T_TILE, loop=1, gi=G_IN, go=G_OUT, xbufs=XBUFS,
                obufs=OBUFS, opbufs=OPBUFS, inq=INQ, outq=OUTQ,
                epi_split=EPI_SPLIT, order=ORDER, probe=None,
                z1bufs=Z1BUFS, pewarm=0):
    assert d_in % 128 == 0 and d_out % 128 == 0 and tok % t_tile == 0
    assert r == 128 and t_tile <= 512
    nk, no, nt = d_in // 128, d_out // 128, tok // t_tile
    assert nk % gi == 0 and no % go == 0

    nc = bacc.Bacc("TRN2", target_bir_lowering=False, debug=False)

    xt = nc.dram_tensor("xt", [nt, nk // gi, 128, gi, t_tile], BF16,
                        kind="ExternalInput")
    outt = nc.dram_tensor("outt", [nt, no // go, 128, go, t_tile], BF16,
                          kind="ExternalOutput")
    w1 = nc.dram_tensor("w1", [128, nk, r], BF16, kind="ExternalInput")
    w2 = nc.dram_tensor("w2", [r, d_out], BF16, kind="ExternalInput")
    s1c = nc.dram_tensor("s1c", [128, no], F32, kind="ExternalInput")
    biasc = nc.dram_tensor("biasc", [128, no], F32, kind="ExternalInput")

    in_dma = _q_cycler(nc, "sp,pool" if inq == "blk" else inq)
    out_dma = _q_cycler(nc, outq)

    with tile.TileContext(nc) as tc, ExitStack() as ctx:
        const = ctx.enter_context(tc.tile_pool(name="const", bufs=1))
        xpool = ctx.enter_context(tc.tile_pool(name="x", bufs=xbufs))
        z1s = ctx.enter_context(tc.tile_pool(name="z1s", bufs=2))
        osb = ctx.enter_context(tc.tile_pool(name="osb", bufs=obufs))
        z1pool = ctx.enter_context(
            tc.tile_pool(name="z1p", bufs=z1bufs, space="PSUM"))
        opsum = ctx.enter_context(
            tc.tile_pool(name="opsum", bufs=opbufs, space="PSUM"))

        w1_sb = const.tile([128, nk, r], BF16)
        nc.sync.dma_start(w1_sb[:], w1.ap())
        w2_sb = const.tile([128, d_out], BF16)
        nc.sync.dma_start(w2_sb[:], w2.ap())
        s1_sb = const.tile([128, no], F32)
        nc.sync.dma_start(s1_sb[:], s1c.ap())
        b_sb = const.tile([128, no], F32)
        nc.sync.dma_start(b_sb[:], biasc.ap())

        if probe == "dma":
            # Pure-streaming roofline probe: same in/out DMA pattern, no
            # compute. ob0 is a constant source for all output DMAs.
            ob0 = const.tile([128, go, t_tile], BF16)
            nc.vector.memset(ob0[:], 0.0)
            if loop > 1:
                ctx.enter_context(
                    tc.For_i(0, loop, 1, hint_engines=LOOP_HINTS))
            for t in range(nt):
                for kg in range(nk // gi):
                    xk = xpool.tile([128, gi, t_tile], BF16)
                    in_dma().dma_start(xk[:], xt.ap()[t, kg])
                for og in range(no // go):
                    out_dma().dma_start(outt.ap()[t, og], ob0[:])

        if loop > 1 and probe != "dma":
            ctx.enter_context(
                tc.For_i(0, loop, 1, hint_engines=LOOP_HINTS))

        def stage2_block(t, z1b):
            for og in range(no // go):
                ob = osb.tile([128, go, t_tile], BF16)
                for oi in range(go):
                    o = og * go + oi
                    op = opsum.tile([128, t_tile], F32)
                    if og == 0 and oi == 0:
                        # p-state bridge: dep-free dummy matmuls keep PE
                        # busy through the z1-copy gap so it holds its
                        # ramped clock; the real matmul below resets the
                        # accumulator with start=True.
                        for _ in range(pewarm):
                            nc.tensor.matmul(op[:], w2_sb[:, 0:128],
                                             w2_sb[:, :t_tile],
                                             start=True, stop=False)
                    nc.tensor.matmul(
                        op[:], w2_sb[:, o * 128:(o + 1) * 128], z1b[:],
                        start=True, stop=True)
                    if epi_split == "half":
                        # column-split each tile ACT/DVE: halves per-tile
                        # epilogue latency so PSUM banks free sooner
                        h = t_tile // 2
                        nc.scalar.activation(ob[:, oi, :h], op[:, :h],
                                             Ident, bias=b_sb[:, o:o + 1],
                                             scale=s1_sb[:, o:o + 1])
                        nc.vector.tensor_scalar(ob[:, oi, h:], op[:, h:],
                                                s1_sb[:, o:o + 1],
                                                b_sb[:, o:o + 1], MULT, ADD)
                        continue
                    if epi_split == "alt3":
                        eng = ("act", "dve", "pool")[og % 3]
                    elif epi_split == "alt":
                        eng = ("act", "dve")[og % 2]
                    elif epi_split == "w53":
                        # ACT is 1.2 GHz vs DVE 0.96, and DVE also does
                        # the z1 copy -> give ACT 5 of every 8 groups
                        eng = "act" if og % 8 in (0, 2, 4, 5, 7) else "dve"
                    elif epi_split == "w35":
                        # ACT also issues all 8 out-DMAs (~5.3 us/tile of
                        # SEQ time), so it is the heavier engine under
                        # "alt" -> give ACT only 3 of every 8 groups
                        eng = "act" if og % 8 in (0, 3, 6) else "dve"
                    elif epi_split == "altoi":
                        # per-tile alternation: both engines work each
                        # group in parallel so its out-DMA issues sooner
                        eng = ("act", "dve")[(og * go + oi) % 2]
                    else:
                        eng = epi_split
                    if eng == "act":
                        nc.scalar.activation(ob[:, oi, :], op[:], Ident,
                                             bias=b_sb[:, o:o + 1],
                                             scale=s1_sb[:, o:o + 1])
                    elif eng == "pool":
                        nc.gpsimd.tensor_scalar(ob[:, oi, :], op[:],
                                                s1_sb[:, o:o + 1],
                                                b_sb[:, o:o + 1], MULT, ADD)
                    else:
                        nc.vector.tensor_scalar(ob[:, oi, :], op[:],
                                                s1_sb[:, o:o + 1],
                                                b_sb[:, o:o + 1], MULT, ADD)
                out_dma().dma_start(outt.ap()[t, og], ob[:])

        pending = None
        for t in range(nt if probe != "dma" else 0):
            z1p = z1pool.tile([128, t_tile], F32)
            xg = {}
            ngr = nk // gi
            for kg in range(ngr):
                xk = xpool.tile([128, gi, t_tile], BF16)
                if inq == "blk":
                    # PE consumes chunks in order: serve the first half of
                    # each tile from the fast SP HWDGE queue, prefetch the
                    # second half on the Pool SWDGE queue whose gen latency
                    # is then hidden behind the earlier chunks.
                    eng = nc.sync if kg < ngr // 2 else nc.gpsimd
                else:
                    eng = in_dma()
                eng.dma_start(xk[:], xt.ap()[t, kg])
                xg[kg] = xk
            for k in range(nk):
                xk = xg[k // gi][:, k % gi, :]
                nc.tensor.matmul(z1p[:], w1_sb[:, k, :], xk,
                                 start=(k == 0), stop=(k == nk - 1))

            z1b = z1s.tile([128, t_tile], BF16, tag="z1b")
            nc.vector.tensor_copy(z1b[:], z1p[:])

            if order == "defer":
                if pending is not None:
                    stage2_block(*pending)
                pending = (t, z1b)
            else:
                stage2_block(t, z1b)
        if pending is not None:
            stage2_block(*pending)

    nc.compile()
    return nc


def build_bf16x2h(d_in=D_IN, d_out=D_OUT, r=R, tok=TOK_PER_CORE,
                  t_tile=512, loop=1, g=4, xbufs=5):
    """Previous baseline: bf16 hi/lo input pair, f32 output."""
    nk, no, nt = d_in // 128, d_out // 128, tok // t_tile

    nc = bacc.Bacc("TRN2", target_bir_lowering=False, debug=False)

    xt = nc.dram_tensor("xt", [nt, nk, 128, t_tile], BF16,
                        kind="ExternalInput")
    xt2 = nc.dram_tensor("xt2", [nt, nk, 128, t_tile], BF16,
                         kind="ExternalInput")
    outt = nc.dram_tensor("outt", [nt, no, 128, t_tile], F32,
                          kind="ExternalOutput")
    w1 = nc.dram_tensor("w1", [128, nk, r], BF16, kind="ExternalInput")
    w2 = nc.dram_tensor("w2", [r, d_out], BF16, kind="ExternalInput")
    s1c = nc.dram_tensor("s1c", [128, no], F32, kind="ExternalInput")
    biasc = nc.dram_tensor("biasc", [128, no], F32, kind="ExternalInput")

    _rr = [0]

    def _dma():
        _rr[0] += 1
        return nc.sync if _rr[0] % 2 else nc.gpsimd

    with tile.TileContext(nc) as tc, ExitStack() as ctx:
        const = ctx.enter_context(tc.tile_pool(name="const", bufs=1))
        xpool = ctx.enter_context(tc.tile_pool(name="x", bufs=xbufs))
        z1s = ctx.enter_context(tc.tile_pool(name="z1s", bufs=2))
        osb = ctx.enter_context(tc.tile_pool(name="osb", bufs=3))
        z1pool = ctx.enter_context(
            tc.tile_pool(name="z1p", bufs=z1bufs, space="PSUM"))
        opsum = ctx.enter_context(
            tc.tile_pool(name="opsum", bufs=4, space="PSUM"))

        w1_sb = const.tile([128, nk, r], BF16)
        nc.sync.dma_start(w1_sb[:], w1.ap())
        w2_sb = const.tile([128, d_out], BF16)
        nc.sync.dma_start(w2_sb[:], w2.ap())
        s1_sb = const.tile([128, no], F32)
        nc.sync.dma_start(s1_sb[:], s1c.ap())
        b_sb = const.tile([128, no], F32)
        nc.sync.dma_start(b_sb[:], biasc.ap())

        if loop > 1:
            ctx.enter_context(
                tc.For_i(0, loop, 1, hint_engines=LOOP_HINTS))

        for t in range(nt):
            z1p = z1pool.tile([128, t_tile], F32)
            xg, xg2 = {}, {}
            for kg in range(nk // g):
                xk = xpool.tile([128, g, t_tile], BF16)
                _dma().dma_start(
                    xk[:], xt.ap()[t, kg * g:(kg + 1) * g].rearrange(
                        "g p s -> p g s"))
                xg[kg] = xk
                xk2 = xpool.tile([128, g, t_tile], BF16, tag="xk2",
                                 name="xk2")
                _dma().dma_start(
                    xk2[:], xt2.ap()[t, kg * g:(kg + 1) * g].rearrange(
                        "g p s -> p g s"))
                xg2[kg] = xk2
            for k in range(nk):
                xk = xg[k // g][:, k % g, :]
                xk2 = xg2[k // g][:, k % g, :]
                nc.tensor.matmul(z1p[:], w1_sb[:, k, :], xk,
                                 start=(k == 0), stop=False)
                nc.tensor.matmul(z1p[:], w1_sb[:, k, :], xk2,
                                 start=False, stop=(k == nk - 1))

            z1hi = z1s.tile([128, t_tile], BF16, tag="z1hi")
            nc.vector.tensor_copy(z1hi[:], z1p[:])
            z1lo = z1s.tile([128, t_tile], BF16, tag="z1lo")
            nc.vector.tensor_tensor(z1lo[:], z1p[:], z1hi[:], SUB)

            for og in range(no // g):
                ob = osb.tile([128, g, t_tile], F32)
                for oi in range(g):
                    o = og * g + oi
                    op = opsum.tile([128, t_tile], F32)
                    nc.tensor.matmul(
                        op[:], w2_sb[:, o * 128:(o + 1) * 128], z1hi[:],
                        start=True, stop=False)
                    nc.tensor.matmul(
                        op[:], w2_sb[:, o * 128:(o + 1) * 128], z1lo[:],
                        start=False, stop=True)
                    nc.scalar.activation(ob[:, oi, :], op[:], Ident,
                                         bias=b_sb[:, o:o + 1],
                                         scale=s1_sb[:, o:o + 1])
                _dma().dma_start(
                    outt.ap()[t, og * g:(og + 1) * g].rearrange(
                        "g p s -> p g s"), ob[:])

    nc.compile()
    return nc


def build_nc(mode=MODE, **kw):
    if mode == "b16io":
        return build_b16io(**kw)
    if mode == "bf16x2h":
        return build_bf16x2h(**kw)
    raise ValueError(mode)


def prep_inputs(x, U_latent, V_latent, s1, s2, bias, mode=MODE,
                n_cores=N_CORES, t_tile=T_TILE, gi=G_IN):
    """Host-side prep: fold s2 into x, sign + cast factors, shard tokens."""
    import ml_dtypes

    tokens = x.shape[0] * x.shape[1] if x.ndim == 3 else x.shape[0]
    d_in = x.shape[-1]
    tok_pc = tokens // n_cores
    nt, nk = tok_pc // t_tile, d_in // 128

    x2 = x.reshape(tokens, d_in) * s2[None, :]
    w1 = np.sign(V_latent).astype(np.float32)
    # pack [d_in, r] -> [128, nk, r] so the SBUF upload is contiguous
    w1 = np.ascontiguousarray(
        w1.reshape(nk, 128, -1).transpose(1, 0, 2)).astype(ml_dtypes.bfloat16)
    w2 = np.ascontiguousarray(
        np.sign(U_latent).astype(np.float32).T).astype(ml_dtypes.bfloat16)
    no = w2.shape[1] // 128
    s1c = np.ascontiguousarray(s1.reshape(no, 128).T)
    biasc = np.ascontiguousarray(bias.reshape(no, 128).T)

    if mode == "b16io":
        xb = x2.astype(ml_dtypes.bfloat16)

        def tilefmt(c):
            xs = xb[c * tok_pc:(c + 1) * tok_pc, :]
            # [nt, T, nk/gi, gi, 128] -> [nt, nk/gi, 128, gi, T]:
            # fully contiguous per DMA tile
            return np.ascontiguousarray(
                xs.reshape(nt, t_tile, nk // gi, gi, 128).transpose(
                    0, 2, 4, 3, 1))

        return [{"w1": w1, "w2": w2, "s1c": s1c, "biasc": biasc,
                 "xt": tilefmt(c)} for c in range(n_cores)]

    xhi = x2.astype(ml_dtypes.bfloat16)
    xlo = (x2 - xhi.astype(np.float32)).astype(ml_dtypes.bfloat16)

    def tilefmt2(arr2d, c):
        xs = arr2d[c * tok_pc:(c + 1) * tok_pc, :]
        # [nt, T, nk, 128] -> [nt, nk, 128, T]
        return np.ascontiguousarray(
            xs.reshape(nt, t_tile, nk, 128).transpose(0, 2, 3, 1))

    return [{"w1": w1, "w2": w2, "s1c": s1c, "biasc": biasc,
             "xt": tilefmt2(xhi, c), "xt2": tilefmt2(xlo, c)}
            for c in range(n_cores)]


def gather_out(results, mode=MODE, n_cores=N_CORES, t_tile=T_TILE, go=G_OUT):
    out = np.empty((TOKENS, D_OUT), np.float32)
    for c in range(n_cores):
        ot = results[c]["outt"]
        if mode == "b16io":
            # [nt, no/go, 128, go, T] -> [tok_pc, d_out], bf16 -> f32
            shard = ot.transpose(0, 4, 1, 3, 2).reshape(
                TOK_PER_CORE, D_OUT).astype(np.float32)
        else:
            # [nt, no, 128, T] -> [tok_pc, d_out]
            shard = ot.transpose(0, 3, 1, 2).reshape(TOK_PER_CORE, D_OUT)
        out[c * TOK_PER_CORE:(c + 1) * TOK_PER_CORE, :] = shard
    return out.reshape(B, S, D_OUT)


_NC_CACHE = {}


def run(inputs, mode=MODE, trace=False):
    if mode not in _NC_CACHE:
        _NC_CACHE[mode] = build_nc(mode=mode)
    nc = _NC_CACHE[mode]
    in_maps = prep_inputs(**inputs, mode=mode)
    res = run_bass_kernel_spmd(nc, in_maps, list(range(N_CORES)),
                               trace=trace)
    return gather_out(res.results, mode=mode), res


def kernel(**inputs):
    inputs = {k: np.asarray(v) for k, v in inputs.items()}
    out, _ = run(inputs)
    return out


# revision 24
# speedup vs baseline: 1.0830x; 1.0370x over previous
"""BinaryFactoredLinear Trainium2 kernel.

Computes out = ((x * s2) @ sign(V)) @ sign(U).T * s1 + bias for
x [4, 4096, 4096] f32, factors [4096, 128] / [4096] — token-sharded
across 8 NeuronCores (2048 tokens each), run SPMD via
run_bass_kernel_spmd.

Default mode "b16io" (memory-bound problem -> halve HBM traffic):
host folds s2 into x (exact f32) and rounds once to bf16; the sign
matrices are +-1 so bf16 weights are exact. The kernel streams bf16
x-tiles in, accumulates both matmul stages in f32 PSUM, applies
s1/bias (exact f32 on-chip) during the PSUM->SBUF copy, and writes
bf16 outputs which the host upcasts to f32. Per-core HBM traffic is
16.8 MiB in + 16.8 MiB out (vs 33.5+33.5 for the bf16x2h mode).
Rounding error: x2->bf16, z1->bf16, out->bf16, each ~2^-9 relative;
measured end-to-end max rel err ~1e-3 against the f32 reference
(gate is 2e-2).

Host pre-tiles each core's token shard into contiguous [128, g*T]
blocks ("fat" layout) so every DMA is one fully contiguous 512 KiB+
transfer with the contraction dim on SBUF partitions. Outputs are
written the same way and reassembled on host.

Per-core pipeline (tokens tiled by T=512, all matmuls N=512 bf16):
  stage 1: z1[r=128, T] += V_sign_k.T @ x_k (32 k-chunks, one PSUM bank)
  z1 -> bf16 SBUF copy (DVE)
  stage 2: out[o*128:(o+1)*128, T] = U_sign_o @ z1
  epilogue: scale*x+bias fused into the PSUM->SBUF copy, alternating
            ScalarE activation / DVE tensor_scalar per TILE (altoi)
            so both engines finish each o-group in parallel and its
            out-DMA issues ~0.9 us sooner (106.3 vs 107.6 us for
            per-group alternation, same-process control).
DMA queues: inputs round-robin over the SP HWDGE + gpsimd SWDGE
queues, outputs on the ACT HWDGE queue.

Tuned via For_i trip-count-diff timing on the 8 cores (see
loopbench2.py / sweep2.py): deep buffering is the lever — xbufs=16
input tiles in flight (2 full t-tiles, rotation-aligned), obufs=8
output tiles (1 t-tile of groups), 6 PSUM banks for stage-2 plus 2
for z1. With the per-tile epilogue split draining banks on both
engines, 6 stage-2 banks suffice and the z1 double-buffer wins the
8th bank back (102.7 vs 105.7 us same-process); under per-group
alternation the trade went the other way (7+1 beat 6+2).
Measured 108.1 us/iter vs a 100.5 us pure-DMA-streaming probe of the
same traffic (334 GB/s/core) and 261.5 us for the bf16x2h baseline.
Rejected by measurement: deferred stage-2 ordering (152 us), ACT-only
epilogue (162), t_tile=256 (136), go=8 (122), gi=2 (113), 3-queue
spread (135), column-split epilogue (118), 5:3 ACT-weighted split
(116). gpsimd.tensor_scalar does not lower on this stack.

Mode "bf16x2h" (previous baseline, rel err ~3.5e-6): x2 split into
bf16 hi/lo pair carrying ~16 mantissa bits, f32 output.
"""

import os
from contextlib import ExitStack

import numpy as np

import concourse.bacc as bacc
import concourse.mybir as mybir
import concourse.tile as tile
from concourse.bass_utils import run_bass_kernel_spmd

F32 = mybir.dt.float32
BF16 = mybir.dt.bfloat16

B, S, D_IN, D_OUT, R = 4, 4096, 4096, 4096, 128
N_CORES = 8
TOKENS = B * S
TOK_PER_CORE = TOKENS // N_CORES

MODE = os.environ.get("BFL_MODE", "b16io")
T_TILE = int(os.environ.get("BFL_T_TILE", "512"))
G_IN = int(os.environ.get("BFL_G_IN", "4"))
G_OUT = int(os.environ.get("BFL_G_OUT", "4"))
XBUFS = int(os.environ.get("BFL_XBUFS", "16"))
OBUFS = int(os.environ.get("BFL_OBUFS", "8"))
OPBUFS = int(os.environ.get("BFL_OPBUFS", "6"))
INQ = os.environ.get("BFL_INQ", "sp,pool")
OUTQ = os.environ.get("BFL_OUTQ", "act")
EPI_SPLIT = os.environ.get("BFL_EPI_SPLIT", "altoi")  # altoi | alt | act | dve
ORDER = os.environ.get("BFL_ORDER", "inline")  # inline | defer
Z1BUFS = int(os.environ.get("BFL_Z1BUFS", "2"))

Copy = mybir.ActivationFunctionType.Copy
Ident = mybir.ActivationFunctionType.Identity
SUB = mybir.AluOpType.subtract
MULT = mybir.AluOpType.mult
ADD = mybir.AluOpType.add

LOOP_HINTS = (mybir.EngineType.PE, mybir.EngineType.DVE,
              mybir.EngineType.Activation, mybir.EngineType.SP,
              mybir.EngineType.Pool)


def _q_cycler(nc, spec):
    ifaces = {"sp": nc.sync, "act": nc.scalar, "pool": nc.gpsimd}
    seq = [ifaces[s.strip()] for s in spec.split(",")]
    state = [0]

    def nxt():
        e = seq[state[0] % len(seq)]
        state[0] += 1
        return e
    return nxt


def build_b16io(d_in=D_IN, d_out=D_OUT, r=R, tok=TOK_PER_CORE,
                t_tile=T_TILE, loop=1, gi=G_IN, go=G_OUT, xbufs=XBUFS,
                obufs=OBUFS, opbufs=OPBUFS, inq=INQ, outq=OUTQ,
                epi_split=EPI_SPLIT, order=ORDER, probe=None,
                z1bufs=Z1BUFS, pewarm=0):
    assert d_in % 128 == 0 and d_out % 128 == 0 and tok % t_tile == 0
    assert r == 128 and t_tile <= 512
    nk, no, nt = d_in // 128, d_out // 128, tok // t_tile
    assert nk % gi == 0 and no % go == 0

    nc = bacc.Bacc("TRN2", target_bir_lowering=False, debug=False)

    xt = nc.dram_tensor("xt", [nt, nk // gi, 128, gi, t_tile], BF16,
                        kind="ExternalInput")
    outt = nc.dram_tensor("outt", [nt, no // go, 128, go, t_tile], BF16,
                          kind="ExternalOutput")
    w1 = nc.dram_tensor("w1", [128, nk, r], BF16, kind="ExternalInput")
    w2 = nc.dram_tensor("w2", [r, d_out], BF16, kind="ExternalInput")
    s1c = nc.dram_tensor("s1c", [128, no], F32, kind="ExternalInput")
    biasc = nc.dram_tensor("biasc", [128, no], F32, kind="ExternalInput")

    in_dma = _q_cycler(nc, "sp,pool" if inq == "blk" else inq)
    out_dma = _q_cycler(nc, outq)

    with tile.TileContext(nc) as tc, ExitStack() as ctx:
        const = ctx.enter_context(tc.tile_pool(name="const", bufs=1))
        xpool = ctx.enter_context(tc.tile_pool(name="x", bufs=xbufs))
        z1s = ctx.enter_context(tc.tile_pool(name="z1s", bufs=2))
        osb = ctx.enter_context(tc.tile_pool(name="osb", bufs=obufs))
        z1pool = ctx.enter_context(
            tc.tile_pool(name="z1p", bufs=z1bufs, space="PSUM"))
        opsum = ctx.enter_context(
            tc.tile_pool(name="opsum", bufs=opbufs, space="PSUM"))

        w1_sb = const.tile([128, nk, r], BF16)
        nc.sync.dma_start(w1_sb[:], w1.ap())
        w2_sb = const.tile([128, d_out], BF16)
        nc.sync.dma_start(w2_sb[:], w2.ap())
        s1_sb = const.tile([128, no], F32)
        nc.sync.dma_start(s1_sb[:], s1c.ap())
        b_sb = const.tile([128, no], F32)
        nc.sync.dma_start(b_sb[:], biasc.ap())

        if probe == "dma":
            # Pure-streaming roofline probe: same in/out DMA pattern, no
            # compute. ob0 is a constant source for all output DMAs.
            ob0 = const.tile([128, go, t_tile], BF16)
            nc.vector.memset(ob0[:], 0.0)
            if loop > 1:
                ctx.enter_context(
                    tc.For_i(0, loop, 1, hint_engines=LOOP_HINTS))
            for t in range(nt):
                for kg in range(nk // gi):
                    xk = xpool.tile([128, gi, t_tile], BF16)
                    in_dma().dma_start(xk[:], xt.ap()[t, kg])
                for og in range(no // go):
                    out_dma().dma_start(outt.ap()[t, og], ob0[:])

        if loop > 1 and probe != "dma":
            ctx.enter_context(
                tc.For_i(0, loop, 1, hint_engines=LOOP_HINTS))

        def stage2_block(t, z1b):
            for og in range(no // go):
                ob = osb.tile([128, go, t_tile], BF16)
                for oi in range(go):
                    o = og * go + oi
                    op = opsum.tile([128, t_tile], F32)
                    if og == 0 and oi == 0:
                        # p-state bridge: dep-free dummy matmuls keep PE
                        # busy through the z1-copy gap so it holds its
                        # ramped clock; the real matmul below resets the
                        # accumulator with start=True.
                        for _ in range(pewarm):
                            nc.tensor.matmul(op[:], w2_sb[:, 0:128],
                                             w2_sb[:, :t_tile],
                                             start=True, stop=False)
                    nc.tensor.matmul(
                        op[:], w2_sb[:, o * 128:(o + 1) * 128], z1b[:],
                        start=True, stop=True)
                    if epi_split == "half":
                        # column-split each tile ACT/DVE: halves per-tile
                        # epilogue latency so PSUM banks free sooner
                        h = t_tile // 2
                        nc.scalar.activation(ob[:, oi, :h], op[:, :h],
                                             Ident, bias=b_sb[:, o:o + 1],
                                             scale=s1_sb[:, o:o + 1])
                        nc.vector.tensor_scalar(ob[:, oi, h:], op[:, h:],
                                                s1_sb[:, o:o + 1],
                                                b_sb[:, o:o + 1], MULT, ADD)
                        continue
                    if epi_split == "alt3":
                        eng = ("act", "dve", "pool")[og % 3]
                    elif epi_split == "alt":
                        eng = ("act", "dve")[og % 2]
                    elif epi_split == "w53":
                        # ACT is 1.2 GHz vs DVE 0.96, and DVE also does
                        # the z1 copy -> give ACT 5 of every 8 groups
                        eng = "act" if og % 8 in (0, 2, 4, 5, 7) else "dve"
                    elif epi_split == "w35":
                        # ACT also issues all 8 out-DMAs (~5.3 us/tile of
                        # SEQ time), so it is the heavier engine under
                        # "alt" -> give ACT only 3 of every 8 groups
                        eng = "act" if og % 8 in (0, 3, 6) else "dve"
                    elif epi_split == "altoi":
                        # per-tile alternation: both engines work each
                        # group in parallel so its out-DMA issues sooner
                        eng = ("act", "dve")[(og * go + oi) % 2]
                    else:
                        eng = epi_split
                    if eng == "act":
                        nc.scalar.activation(ob[:, oi, :], op[:], Ident,
                                             bias=b_sb[:, o:o + 1],
                                             scale=s1_sb[:, o:o + 1])
                    elif eng == "pool":
                        nc.gpsimd.tensor_scalar(ob[:, oi, :], op[:],
                                                s1_sb[:, o:o + 1],
                                                b_sb[:, o:o + 1], MULT, ADD)
                    else:
                        nc.vector.tensor_scalar(ob[:, oi, :], op[:],
                                                s1_sb[:, o:o + 1],
                                                b_sb[:, o:o + 1], MULT, ADD)
                out_dma().dma_start(outt.ap()[t, og], ob[:])

        pending = None
        for t in range(nt if probe != "dma" else 0):
            z1p = z1pool.tile([128, t_tile], F32)
            xg = {}
            ngr = nk // gi
            for kg in range(ngr):
                xk = xpool.tile([128, gi, t_tile], BF16)
                if inq == "blk":
                    # PE consumes chunks in order: serve the first half of
                    # each tile from the fast SP HWDGE queue, prefetch the
                    # second half on the Pool SWDGE queue whose gen latency
                    # is then hidden behind the earlier chunks.
                    eng = nc.sync if kg < ngr // 2 else nc.gpsimd
                else:
                    eng = in_dma()
                eng.dma_start(xk[:], xt.ap()[t, kg])
                xg[kg] = xk
            for k in range(nk):
                xk = xg[k // gi][:, k % gi, :]
                nc.tensor.matmul(z1p[:], w1_sb[:, k, :], xk,
                                 start=(k == 0), stop=(k == nk - 1))

            z1b = z1s.tile([128, t_tile], BF16, tag="z1b")
            nc.vector.tensor_copy(z1b[:], z1p[:])

            if order == "defer":
                if pending is not None:
                    stage2_block(*pending)
                pending = (t, z1b)
            else:
                stage2_block(t, z1b)
        if pending is not None:
            stage2_block(*pending)

    nc.compile()
    return nc


def build_bf16x2h(d_in=D_IN, d_out=D_OUT, r=R, tok=TOK_PER_CORE,
                  t_tile=512, loop=1, g=4, xbufs=5):
    """Previous baseline: bf16 hi/lo input pair, f32 output."""
    nk, no, nt = d_in // 128, d_out // 128, tok // t_tile

    nc = bacc.Bacc("TRN2", target_bir_lowering=False, debug=False)

    xt = nc.dram_tensor("xt", [nt, nk, 128, t_tile], BF16,
                        kind="ExternalInput")
    xt2 = nc.dram_tensor("xt2", [nt, nk, 128, t_tile], BF16,
                         kind="ExternalInput")
    outt = nc.dram_tensor("outt", [nt, no, 128, t_tile], F32,
                          kind="ExternalOutput")
    w1 = nc.dram_tensor("w1", [128, nk, r], BF16, kind="ExternalInput")
    w2 = nc.dram_tensor("w2", [r, d_out], BF16, kind="ExternalInput")
    s1c = nc.dram_tensor("s1c", [128, no], F32, kind="ExternalInput")
    biasc = nc.dram_tensor("biasc", [128, no], F32, kind="ExternalInput")

    _rr = [0]

    def _dma():
        _rr[0] += 1
        return nc.sync if _rr[0] % 2 else nc.gpsimd

    with tile.TileContext(nc) as tc, ExitStack() as ctx:
        const = ctx.enter_context(tc.tile_pool(name="const", bufs=1))
        xpool = ctx.enter_context(tc.tile_pool(name="x", bufs=xbufs))
        z1s = ctx.enter_context(tc.tile_pool(name="z1s", bufs=2))
        osb = ctx.enter_context(tc.tile_pool(name="osb", bufs=3))
        z1pool = ctx.enter_context(
            tc.tile_pool(name="z1p", bufs=z1bufs, space="PSUM"))
        opsum = ctx.enter_context(
            tc.tile_pool(name="opsum", bufs=4, space="PSUM"))

        w1_sb = const.tile([128, nk, r], BF16)
        nc.sync.dma_start(w1_sb[:], w1.ap())
        w2_sb = const.tile([128, d_out], BF16)
        nc.sync.dma_start(w2_sb[:], w2.ap())
        s1_sb = const.tile([128, no], F32)
        nc.sync.dma_start(s1_sb[:], s1c.ap())
        b_sb = const.tile([128, no], F32)
        nc.sync.dma_start(b_sb[:], biasc.ap())

        if loop > 1:
            ctx.enter_context(
                tc.For_i(0, loop, 1, hint_engines=LOOP_HINTS))

        for t in range(nt):
            z1p = z1pool.tile([128, t_tile], F32)
            xg, xg2 = {}, {}
            for kg in range(nk // g):
                xk = xpool.tile([128, g, t_tile], BF16)
                _dma().dma_start(
                    xk[:], xt.ap()[t, kg * g:(kg + 1) * g].rearrange(
                        "g p s -> p g s"))
                xg[kg] = xk
                xk2 = xpool.tile([128, g, t_tile], BF16, tag="xk2",
                                 name="xk2")
                _dma().dma_start(
                    xk2[:], xt2.ap()[t, kg * g:(kg + 1) * g].rearrange(
                        "g p s -> p g s"))
                xg2[kg] = xk2
            for k in range(nk):
                xk = xg[k // g][:, k % g, :]
                xk2 = xg2[k // g][:, k % g, :]
                nc.tensor.matmul(z1p[:], w1_sb[:, k, :], xk,
                                 start=(k == 0), stop=False)
                nc.tensor.matmul(z1p[:], w1_sb[:, k, :], xk2,
                                 start=False, stop=(k == nk - 1))

            z1hi = z1s.tile([128, t_tile], BF16, tag="z1hi")
            nc.vector.tensor_copy(z1hi[:], z1p[:])
            z1lo = z1s.tile([128, t_tile], BF16, tag="z1lo")
            nc.vector.tensor_tensor(z1lo[:], z1p[:], z1hi[:], SUB)

            for og in range(no // g):
                ob = osb.tile([128, g, t_tile], F32)
                for oi in range(g):
                    o = og * g + oi
                    op = opsum.tile([128, t_tile], F32)
                    nc.tensor.matmul(
                        op[:], w2_sb[:, o * 128:(o + 1) * 128], z1hi[:],
                        start=True, stop=False)
                    nc.tensor.matmul(
                        op[:], w2_sb[:, o * 128:(o + 1) * 128], z1lo[:],
                        start=False, stop=True)
                    nc.scalar.activation(ob[:, oi, :], op[:], Ident,
                                         bias=b_sb[:, o:o + 1],
                                         scale=s1_sb[:, o:o + 1])
                _dma().dma_start(
                    outt.ap()[t, og * g:(og + 1) * g].rearrange(
                        "g p s -> p g s"), ob[:])

    nc.compile()
    return nc


def build_nc(mode=MODE, **kw):
    if mode == "b16io":
        return build_b16io(**kw)
    if mode == "bf16x2h":
        return build_bf16x2h(**kw)
    raise ValueError(mode)


def prep_inputs(x, U_latent, V_latent, s1, s2, bias, mode=MODE,
                n_cores=N_CORES, t_tile=T_TILE, gi=G_IN):
    """Host-side prep: fold s2 into x, sign + cast factors, shard tokens."""
    import ml_dtypes

    tokens = x.shape[0] * x.shape[1] if x.ndim == 3 else x.shape[0]
    d_in = x.shape[-1]
    tok_pc = tokens // n_cores
    nt, nk = tok_pc // t_tile, d_in // 128

    x2 = x.reshape(tokens, d_in) * s2[None, :]
    w1 = np.sign(V_latent).astype(np.float32)
    # pack [d_in, r] -> [128, nk, r] so the SBUF upload is contiguous
    w1 = np.ascontiguousarray(
        w1.reshape(nk, 128, -1).transpose(1, 0, 2)).astype(ml_dtypes.bfloat16)
    w2 = np.ascontiguousarray(
        np.sign(U_latent).astype(np.float32).T).astype(ml_dtypes.bfloat16)
    no = w2.shape[1] // 128
    s1c = np.ascontiguousarray(s1.reshape(no, 128).T)
    biasc = np.ascontiguousarray(bias.reshape(no, 128).T)

    if mode == "b16io":
        xb = x2.astype(ml_dtypes.bfloat16)

        def tilefmt(c):
            xs = xb[c * tok_pc:(c + 1) * tok_pc, :]
            # [nt, T, nk/gi, gi, 128] -> [nt, nk/gi, 128, gi, T]:
            # fully contiguous per DMA tile
            return np.ascontiguousarray(
                xs.reshape(nt, t_tile, nk // gi, gi, 128).transpose(
                    0, 2, 4, 3, 1))

        return [{"w1": w1, "w2": w2, "s1c": s1c, "biasc": biasc,
                 "xt": tilefmt(c)} for c in range(n_cores)]

    xhi = x2.astype(ml_dtypes.bfloat16)
    xlo = (x2 - xhi.astype(np.float32)).astype(ml_dtypes.bfloat16)

    def tilefmt2(arr2d, c):
        xs = arr2d[c * tok_pc:(c + 1) * tok_pc, :]
        # [nt, T, nk, 128] -> [nt, nk, 128, T]
        return np.ascontiguousarray(
            xs.reshape(nt, t_tile, nk, 128).transpose(0, 2, 3, 1))

    return [{"w1": w1, "w2": w2, "s1c": s1c, "biasc": biasc,
             "xt": tilefmt2(xhi, c), "xt2": tilefmt2(xlo, c)}
            for c in range(n_cores)]


def gather_out(results, mode=MODE, n_cores=N_CORES, t_tile=T_TILE, go=G_OUT):
    out = np.empty((TOKENS, D_OUT), np.float32)
    for c in range(n_cores):
        ot = results[c]["outt"]
        if mode == "b16io":
            # [nt, no/go, 128, go, T] -> [tok_pc, d_out], bf16 -> f32
            shard = ot.transpose(0, 4, 1, 3, 2).reshape(
                TOK_PER_CORE, D_OUT).astype(np.float32)
        else:
            # [nt, no, 128, T] -> [tok_pc, d_out]
            shard = ot.transpose(0, 3, 1, 2).reshape(TOK_PER_CORE, D_OUT)
        out[c * TOK_PER_CORE:(c + 1) * TOK_PER_CORE, :] = shard
    return out.reshape(B, S, D_OUT)


_NC_CACHE = {}


def run(inputs, mode=MODE, trace=False):
    if mode not in _NC_CACHE:
        _NC_CACHE[mode] = build_nc(mode=mode)
    nc = _NC_CACHE[mode]
    in_maps = prep_inputs(**inputs, mode=mode)
    res = run_bass_kernel_spmd(nc, in_maps, list(range(N_CORES)),
                               trace=trace)
    return gather_out(res.results, mode=mode), res


def kernel(**inputs):
    inputs = {k: np.asarray(v) for k, v in inputs.items()}
    out, _ = run(inputs)
    return out
